# revision 35
# baseline (speedup 1.0000x reference)
"""DensityPooling Trainium2 kernel — ladder-of-squares edition.

Computes, for inputs wrho (B,X), distances (B,X,A), gammas (S,), W (E,S):

    norms_s       = (pi / gammas_s) ** 1.5
    pooled[b,a,s] = sum_x wrho[b,x] * norms_s * exp(-gammas_s * d[b,x,a]^2)
    phi           = log(pooled + eps)
    out[b,a,e]    = sum_s phi[b,a,s] * W[e,s]

Sharding: data-parallel over batch, one batch per NeuronCore (8 cores).

Algorithm (the big win vs the naive S-exp formulation): instead of
evaluating exp(-gamma_s * u) for all 32 gammas (8.4M ACT exps/core,
~55us floor at 1 elem/cycle/lane), evaluate it on a *ratio-2 ladder*
c_j = 2^j spanning the gamma range:

    t_j = exp(-c_j * u),   t_{j+1} = t_j^2        (u = d^2)

Only ceil(J/4) rungs are seeded with a real ACT exp; the rest are DVE
bf16 squarings (2 elem/cycle/lane). Each rung is pooled over x on the
PE (wrho enters as the bf16 stationary operand). The 32 actual gammas
are then reconstructed from the pooled rung values with a tiny [NP,S]
matmul whose coefficients beta come from a host-side least-squares fit
    exp(-g_s u) ~= sum_j beta[j,s] exp(-c_j u)   over u = d^2, d~U[0,dmax]
Pooling integrates the fit residual against exactly the measure the
fit minimizes, so the error cancels to ~1e-5 (measured vs fp32 ref).
Only the best NP<=8 rungs (greedy subset selection) are pooled so the
per-chunk pooling is a single N=NP*64<=512 matmul into one PSUM bank.

Per-core engine budget (vs 82.8us baseline): ACT 4 passes over 262k
elems (u=Square + 3 seed Exps) ~9us, DVE 8 squarings ~10us, PE 32
matmuls of N=512 ~9.5us, DMA-in 1MB ~4us — all overlapped.

Tail: pooled rungs [1,512] -> reshape DMA -> [NP,64]; pooled_s = beta^T
@ grid; phi = Ln(pooled_s + eps/norms) + ln(norms) (norms folded in
log-space: Exp/Ln/Square share one ACT table set, see
_merge_act_table_loads); out[64,256] = phi^T @ W^T.
"""

import math
import os

import numpy as np

import concourse.bacc as bacc
import concourse.bass as bass
import concourse.tile as tile
from concourse import mybir
from concourse.bass_utils import run_bass_kernel_spmd

B, X, A = 8, 4096, 64
S, E = 32, 256
P = 128
C = X // P  # 32 chunks; x = p*C + c
EPS = 1e-4
N_CORES = 8
NPOOL_MAX = int(os.environ.get("DENS_NPOOL", "8"))  # pooled rungs; NP*A <= 512 = one PSUM bank

F32 = mybir.dt.float32
BF16 = mybir.dt.bfloat16
AF = mybir.ActivationFunctionType

SEED_EVERY = int(os.environ.get("DENS_SEED_EVERY", "4"))
GROUPS = [int(g) for g in os.environ.get("DENS_GROUPS", "4,8,8,8,4").split(",")]
# u = d^2 engine (act|vector|gpsimd); gpsimd measured ~5x slower per elem
U_ENGINE = os.environ.get("DENS_U_ENGINE", "act")
# ladder indices whose squaring runs on GPSIMD instead of DVE (chain-tail
# rungs only); gpsimd measured too slow to help, default none
GP_SLOTS = {
    int(s) for s in os.environ.get("DENS_GP_SLOTS", "").split(",") if s != ""
}


# ---------------------------------------------------------------- host math


def _fit_beta(gammas, c_pool, dmax, n_samp=4096, ridge=1e-7):
    """exp(-g u) ~= sum_j beta[j] exp(-c_j u), u=d^2, d uniform [0,dmax]."""
    d = (np.arange(n_samp, dtype=np.float64) + 0.5) / n_samp * dmax
    u = d * d
    Amat = np.exp(-np.outer(u, c_pool))
    Bmat = np.exp(-np.outer(u, gammas))
    scale = Bmat.mean(axis=0)
    AtA = Amat.T @ Amat
    lam = ridge * np.trace(AtA) / len(c_pool)
    beta_n = np.linalg.solve(AtA + lam * np.eye(len(c_pool)), Amat.T @ (Bmat / scale))
    beta = beta_n * scale
    resid = np.abs(Amat @ beta - Bmat).max(axis=0) / scale
    return beta, float(resid.max())


RESID_TOL = float(os.environ.get("DENS_RESID_TOL", "1.5e-3"))


def _needed_set(pool, seeds, J):
    need = set(pool)
    for j in range(J - 1, 0, -1):
        if j in need and j not in seeds:
            need.add(j - 1)
    return need


def _plan(gammas, dmax):
    """Choose ladder, seed rungs, pooled subset, and fit beta.

    Searches seed patterns x pooled subsets minimizing a per-engine cost
    model (ACT: u + seed exps; DVE: squarings; PE: pooled rungs) subject
    to the host-side fit residual staying under RESID_TOL."""
    import itertools

    gammas = np.asarray(gammas, dtype=np.float64)
    gmin, gmax = float(gammas.min()), float(gammas.max())
    assert gmin > 0, "gammas must be positive"
    j_lo = int(np.floor(np.log2(gmin)))
    j_hi = int(np.ceil(np.log2(gmax)))
    j_hi = max(j_hi, j_lo)  # at least one rung
    if j_hi - j_lo > 15:  # cap ladder length; fit covers the rest
        j_lo = j_hi - 15
    c_full = 2.0 ** np.arange(j_lo, j_hi + 1)
    J = len(c_full)

    seed_opts = {tuple(range(0, J, SEED_EVERY))}
    for step in (3, 4):
        seed_opts.add(tuple(sorted({max(0, J - 1 - k * step) for k in range(4)})))
        seed_opts.add(tuple(sorted({max(0, J - 1 - k * step) for k in range(3)})))
    best = None
    np_lo = min(6, NPOOL_MAX)
    for seeds in seed_opts:
        sset = set(seeds)
        for NP in range(np_lo, NPOOL_MAX + 1):
            for pool in itertools.combinations(range(J), min(NP, J)):
                need = _needed_set(pool, sset, J)
                n_seed = len([s for s in seeds if s in need])
                n_sq = len(need) - n_seed
                _, r = _fit_beta(gammas, c_full[list(pool)], dmax, n_samp=256)
                if r > RESID_TOL:
                    continue
                act_t = (1 + n_seed) * 1.95
                dve_t = n_sq * 1.35
                pe_t = len(pool) * 1.35 + 1.2
                score = max(act_t, dve_t, 0.9 * pe_t) + 0.1 * (act_t + dve_t + pe_t)
                key = (score, r)
                if best is None or key < best[0]:
                    best = (key, tuple(pool), tuple(s for s in seeds if s in need))
    assert best is not None, "no ladder subset met the fit tolerance"
    pool = list(best[1])
    seed_idx = list(best[2])
    beta, resid = _fit_beta(gammas, c_full[pool], dmax)
    needed = _needed_set(pool, set(seed_idx), J)
    # slot layout: pooled rungs -> slots 0..NP-1 (ladder order); rest after
    slot_of = {}
    for s_, j in enumerate(pool):
        slot_of[j] = s_
    nxt = len(pool)
    for j in range(J):
        if j not in slot_of:
            slot_of[j] = nxt
            nxt += 1
    return {
        "c_full": tuple(float(c) for c in c_full),
        "seed_idx": tuple(j for j in seed_idx if j in needed),
        "slot_of": tuple(slot_of[j] for j in range(J)),
        "needed": tuple(sorted(needed)),
        "npool": len(pool),
        "beta": beta.astype(np.float32),  # (NP, S)
        "resid": resid,
    }


# ---------------------------------------------------------------- program


def _build_program(c_full, seed_idx, slot_of, npool, needed):
    J = len(c_full)
    NP = npool
    seed_set = set(seed_idx)
    needed = set(needed)
    nc = bacc.Bacc("TRN2", target_bir_lowering=False, debug=False, num_devices=N_CORES)

    d_dram = nc.dram_tensor("d", [X, A], F32, kind="ExternalInput")
    wr_dram = nc.dram_tensor("wr", [X], F32, kind="ExternalInput")
    beta_dram = nc.dram_tensor("beta", [2 * NP * S], F32, kind="ExternalInput")
    wt_dram = nc.dram_tensor("wt", [S + 1, E], F32, kind="ExternalInput")
    epsn_dram = nc.dram_tensor("epsn", [S], F32, kind="ExternalInput")
    y_dram = nc.dram_tensor("y", [A, E], F32, kind="ExternalOutput")

    group_bounds = [0]
    for g in GROUPS:
        group_bounds.append(group_bounds[-1] + g)
    assert group_bounds[-1] == C, f"groups {GROUPS} must sum to {C}"

    with tile.TileContext(nc) as tc:
        with (
            tc.tile_pool(name="singles", bufs=1) as singles,
            tc.tile_pool(name="upool", bufs=int(os.environ.get("DENS_UBUFS", "4"))) as upool,
            tc.tile_pool(name="tpool", bufs=int(os.environ.get("DENS_TBUFS", "4"))) as tpool,
            tc.tile_pool(name="psum", bufs=1, space="PSUM") as psum,
        ):
            # ---- input loads (d pieces per compute group for pipelining;
            # first piece first so group-1 compute starts ASAP) ----
            d_sb = singles.tile([P, C, A], F32)
            d_src = d_dram.ap().rearrange("(p c) a -> p c a", p=P)
            for q in range(0, 2):
                lo, hi = group_bounds[q], group_bounds[q + 1]
                nc.sync.dma_start(out=d_sb[:, lo:hi, :], in_=d_src[:, lo:hi, :])
            wr_sb = singles.tile([P, C], F32)
            nc.sync.dma_start(out=wr_sb[:], in_=wr_dram.ap().rearrange("(p c) -> p c", p=P))
            wr_bf = singles.tile([P, C], BF16)
            nc.vector.tensor_copy(wr_bf[:], wr_sb[:])
            for q in range(2, len(group_bounds) - 1):
                lo, hi = group_bounds[q], group_bounds[q + 1]
                nc.sync.dma_start(out=d_sb[:, lo:hi, :], in_=d_src[:, lo:hi, :])

            # small tail constants behind the d pieces on the sync queue
            # (issuing them from nc.scalar stalls the ACT sequencer mid-loop)
            # beta rows flattened on partition 0: row j at [0, j*S:(j+1)*S]
            beta_sb = singles.tile([1, 2 * NP * S], F32)
            nc.sync.dma_start(out=beta_sb[:], in_=beta_dram.ap().unsqueeze(0))
            wt_sb = singles.tile([S + 1, E], F32)
            nc.sync.dma_start(out=wt_sb[:], in_=wt_dram.ap())
            epsn_sb = singles.tile([S, 1], F32)
            nc.sync.dma_start(out=epsn_sb[:], in_=epsn_dram.ap().unsqueeze(1))

            # ---- main loop: ladder + pooling ----
            # pooling is split: chunks [0, SPLIT) accumulate in psA, whose
            # copy + rank-1 interp updates overlap the last group's ladder;
            # only the small psB part stays on the post-loop critical path.
            # interp: pooled_s[s,a] = sum_j beta[j,s] * pooled[j*A+a] done as
            # 2*NP rank-1 matmuls (lhsT = beta row [1,S], rhs = pooled piece
            # [1,A]) accumulating straight into a [S,A] PSUM tile -- no
            # reshape/transpose of the flat pooled vector is ever needed.
            SPLIT = group_bounds[-2]
            pooled_psA = psum.tile([1, NP * A], F32)
            pooled_psB = psum.tile([1, NP * A], F32)
            pooled_sbA = singles.tile([1, NP * A], F32)
            interp_ps = psum.tile([S, A], F32)
            for q in range(len(group_bounds) - 1):
                c0, c1 = group_bounds[q], group_bounds[q + 1]
                gsz = c1 - c0
                u_g = upool.tile([P, gsz, A], BF16, tag="u")
                if U_ENGINE == "gpsimd":
                    nc.gpsimd.tensor_mul(u_g[:], d_sb[:, c0:c1, :], d_sb[:, c0:c1, :])
                elif U_ENGINE == "vector":
                    nc.vector.tensor_mul(u_g[:], d_sb[:, c0:c1, :], d_sb[:, c0:c1, :])
                else:
                    nc.scalar.activation(u_g[:], d_sb[:, c0:c1, :], AF.Square)
                # chunk-major layout: per-chunk slab [J, A] so the pooling
                # matmul rhs [NP*A] is contiguous (strided rhs costs ~320
                # PE cycles/call extra)
                t_g = tpool.tile([P, gsz, J, A], BF16, tag="t")
                for j in range(J):
                    if j not in needed:
                        continue
                    slot = slot_of[j]
                    if j in seed_set:
                        nc.scalar.activation(
                            t_g[:, :, slot, :], u_g[:], AF.Exp, scale=-c_full[j]
                        )
                    else:
                        prev = slot_of[j - 1]
                        eng = nc.gpsimd if j in GP_SLOTS else nc.vector
                        eng.tensor_mul(
                            t_g[:, :, slot, :], t_g[:, :, prev, :], t_g[:, :, prev, :]
                        )
                if c0 == SPLIT:
                    # A-part tail, issued after the last group's ladder ops
                    # so ACT runs the copy right after its final seeds and PE
                    # runs the interp updates before the (ladder-gated) B pools
                    nc.scalar.copy(pooled_sbA[:], pooled_psA[:])
                    for j in range(NP):
                        nc.tensor.matmul(
                            interp_ps[:],
                            beta_sb[:, j * S : (j + 1) * S],
                            pooled_sbA[:, j * A : (j + 1) * A],
                            start=(j == 0),
                            stop=False,
                            skip_group_check=True,
                        )
                for k in range(gsz):
                    c = c0 + k
                    in_a = c < SPLIT
                    nc.tensor.matmul(
                        pooled_psA[:, :] if in_a else pooled_psB[:, :],
                        wr_bf[:, c : c + 1],
                        t_g[:, k, 0:NP, :],
                        start=(c == 0 or c == SPLIT),
                        stop=(c == SPLIT - 1 or c == C - 1),
                    )

            # ---- tail (B part + combine) ----
            pooled_sbB = singles.tile([1, NP * A], F32)
            half = (NP * A) // 2
            nc.scalar.copy(pooled_sbB[:, :half], pooled_psB[:, :half])
            nc.vector.tensor_copy(pooled_sbB[:, half:], pooled_psB[:, half:])
            for j in range(NP):
                nc.tensor.matmul(
                    interp_ps[:],
                    beta_sb[:, (NP + j) * S : (NP + j + 1) * S],
                    pooled_sbB[:, j * A : (j + 1) * A],
                    start=False,
                    stop=(j == NP - 1),
                    skip_group_check=True,
                )

            # phi = ln(pooled_s + eps/norms); the "+ ln(norms)" term is folded
            # into the final matmul via a constant ones-row in phi and an
            # extra row sum_s ln(norms_s) W[e,s] in wt (host-computed)
            phi = singles.tile([S + 1, A], F32)
            nc.vector.memset(phi[S : S + 1, :], 1.0)
            nc.scalar.activation(
                phi[0:S, :], interp_ps[:], AF.Ln, bias=epsn_sb[:], scale=1.0
            )

            # final lift, split into halves so copy/DMA receipts overlap
            out_ps = psum.tile([A, E], F32)
            out_sb = singles.tile([A, E], F32)
            y_ap = y_dram.ap()
            for h in range(2):
                cs = slice(h * (E // 2), (h + 1) * (E // 2))
                nc.tensor.matmul(
                    out_ps[:, cs], phi[:], wt_sb[:, cs], start=True, stop=True
                )
                if h == 0:
                    nc.scalar.copy(out_sb[:, cs], out_ps[:, cs])
                    nc.sync.dma_start(out=y_ap[:, cs], in_=out_sb[:, cs])
                else:
                    nc.vector.tensor_copy(out_sb[:, cs], out_ps[:, cs])
                    nc.scalar.dma_start(out=y_ap[:, cs], in_=out_sb[:, cs])

    nc.compile()
    _merge_act_table_loads(nc)
    return nc


def _merge_act_table_loads(nc):
    """Exp, Ln and Square all live in the 'natural_log_exp_and_others' set,
    but the table-load pass picks per-function sets, emitting a ~2.7us table
    swap at every transition. Point every load at the combined set and drop
    the redundant reloads (keeping any that carry semaphore waits/updates)."""
    from concourse.hw_specs import get_activation_tables

    tables = list(get_activation_tables(nc.m.arch).items())
    combined_id = None
    for i, (name, funcs) in enumerate(tables):
        if name == "natural_log_exp_and_others":
            combined_id = i
    if combined_id is None:
        return
    needed = {AF.Exp, AF.Ln, AF.Square}
    if not needed <= tables[combined_id][1]:
        return
    for b in nc.main_func.blocks:
        seen = False
        keep = []
        for inst in b.instructions:
            if isinstance(inst, mybir.InstLoadActFuncSet):
                si = inst.sync_info
                has_sync = si is not None and (
                    len(si.on_wait) > 0 or len(si.on_update) > 0
                )
                inst.act_func_set_id = combined_id
                if seen and not has_sync:
                    continue  # redundant reload of the same set
                seen = True
            keep.append(inst)
        if len(keep) != len(b.instructions):
            b.instructions[:] = keep


# ---------------------------------------------------------------- entry


_CACHE = {}


def _get_program_and_plan(gammas, dmax):
    plan = _plan(gammas, dmax)
    key = (
        plan["c_full"], plan["seed_idx"], plan["slot_of"], plan["npool"],
        plan["needed"],
    )
    if key not in _CACHE:
        _CACHE[key] = _build_program(
            plan["c_full"], plan["seed_idx"], plan["slot_of"], plan["npool"],
            plan["needed"],
        )
    return _CACHE[key], plan


def _make_in_maps(wrho, distances, gammas, W, plan):
    wrho = np.ascontiguousarray(np.asarray(wrho, dtype=np.float32))
    distances = np.ascontiguousarray(np.asarray(distances, dtype=np.float32))
    gammas = np.asarray(gammas, dtype=np.float64)
    W = np.asarray(W, dtype=np.float32)
    assert wrho.shape == (B, X) and distances.shape == (B, X, A)
    assert gammas.shape == (S,) and W.shape == (E, S)
    norms = (np.pi / gammas) ** 1.5
    epsn = (EPS / norms).astype(np.float32)
    lnorms = 1.5 * np.log(np.pi / gammas)
    # wt row S carries sum_s ln(norms_s) W[e,s]; paired with a constant
    # ones-row in phi it adds the + ln(norms) term during the final matmul
    wt = np.ascontiguousarray(
        np.vstack([W.T.astype(np.float64), (lnorms @ W.T.astype(np.float64))[None, :]])
    ).astype(np.float32)
    # duplicated rows: the interp rank-1 updates sum the split A/B pooled
    # halves; flattened row-major (row j at [j*S:(j+1)*S])
    beta = np.ascontiguousarray(
        np.vstack([plan["beta"], plan["beta"]]).reshape(-1)
    )
    return [
        {
            "d": distances[b],
            "wr": wrho[b],
            "beta": beta,
            "wt": wt,
            "epsn": epsn,
        }
        for b in range(B)
    ]


def kernel(wrho, distances, gammas, W, **_unused):
    dmax = float(np.abs(np.asarray(distances)).max())
    nc, plan = _get_program_and_plan(gammas, max(dmax, 1e-6))
    in_maps = _make_in_maps(wrho, distances, gammas, W, plan)
    res = run_bass_kernel_spmd(nc, in_maps, core_ids=list(range(N_CORES)))
    return np.stack([res.results[b]["y"] for b in range(B)], axis=0)


def kernel_traced(wrho, distances, gammas, W):
    """Like kernel() but with NTFF tracing; returns (out, BassKernelResults)."""
    dmax = float(np.abs(np.asarray(distances)).max())
    nc, plan = _get_program_and_plan(gammas, max(dmax, 1e-6))
    in_maps = _make_in_maps(wrho, distances, gammas, W, plan)
    res = run_bass_kernel_spmd(nc, in_maps, core_ids=list(range(N_CORES)), trace=True)
    out = np.stack([res.results[b]["y"] for b in range(B)], axis=0)
    return out, res


# revision 41
# speedup vs baseline: 1.0817x; 1.0817x over previous
"""DensityPooling Trainium2 kernel — ladder-of-squares edition.

Computes, for inputs wrho (B,X), distances (B,X,A), gammas (S,), W (E,S):

    norms_s       = (pi / gammas_s) ** 1.5
    pooled[b,a,s] = sum_x wrho[b,x] * norms_s * exp(-gammas_s * d[b,x,a]^2)
    phi           = log(pooled + eps)
    out[b,a,e]    = sum_s phi[b,a,s] * W[e,s]

Sharding: data-parallel over batch, one batch per NeuronCore (8 cores).

Algorithm (the big win vs the naive S-exp formulation): instead of
evaluating exp(-gamma_s * u) for all 32 gammas (8.4M ACT exps/core,
~55us floor at 1 elem/cycle/lane), evaluate it on a *ratio-2 ladder*
c_j = 2^j spanning the gamma range:

    t_j = exp(-c_j * u),   t_{j+1} = t_j^2        (u = d^2)

Only ceil(J/4) rungs are seeded with a real ACT exp; the rest are DVE
bf16 squarings (2 elem/cycle/lane). Each rung is pooled over x on the
PE (wrho enters as the bf16 stationary operand). The 32 actual gammas
are then reconstructed from the pooled rung values with a tiny [NP,S]
matmul whose coefficients beta come from a host-side least-squares fit
    exp(-g_s u) ~= sum_j beta[j,s] exp(-c_j u)   over u = d^2, d~U[0,dmax]
Pooling integrates the fit residual against exactly the measure the
fit minimizes, so the error cancels to ~1e-5 (measured vs fp32 ref).
Only the best NP<=8 rungs (greedy subset selection) are pooled so the
per-chunk pooling is a single N=NP*64<=512 matmul into one PSUM bank.

Per-core engine budget (vs 82.8us baseline): ACT 4 passes over 262k
elems (u=Square + 3 seed Exps) ~9us, DVE 8 squarings ~10us, PE 32
matmuls of N=512 ~9.5us, DMA-in 1MB ~4us — all overlapped.

Tail: pooled rungs [1,512] -> reshape DMA -> [NP,64]; pooled_s = beta^T
@ grid; phi = Ln(pooled_s + eps/norms) + ln(norms) (norms folded in
log-space: Exp/Ln/Square share one ACT table set, see
_merge_act_table_loads); out[64,256] = phi^T @ W^T.
"""

import math
import os

import numpy as np

import concourse.bacc as bacc
import concourse.bass as bass
import concourse.tile as tile
from concourse import mybir
from concourse.bass_utils import run_bass_kernel_spmd

B, X, A = 8, 4096, 64
S, E = 32, 256
P = 128
C = X // P  # 32 chunks; x = p*C + c
EPS = 1e-4
N_CORES = 8
NPOOL_MAX = int(os.environ.get("DENS_NPOOL", "8"))  # pooled rungs; NP*A <= 512 = one PSUM bank

F32 = mybir.dt.float32
BF16 = mybir.dt.bfloat16
AF = mybir.ActivationFunctionType

SEED_EVERY = int(os.environ.get("DENS_SEED_EVERY", "4"))
GROUPS = [int(g) for g in os.environ.get("DENS_GROUPS", "4,8,8,8,4").split(",")]
# u = d^2 engine (auto|act|vector|gpsimd); auto = balance in _plan;
# gpsimd measured ~5x slower per elem
U_ENGINE = os.environ.get("DENS_U_ENGINE", "auto")
# ladder indices whose squaring runs on GPSIMD instead of DVE (chain-tail
# rungs only); gpsimd measured too slow to help, default none
GP_SLOTS = {
    int(s) for s in os.environ.get("DENS_GP_SLOTS", "").split(",") if s != ""
}


# ---------------------------------------------------------------- host math


def _fit_beta(gammas, c_pool, dmax, n_samp=4096, ridge=1e-7):
    """exp(-g u) ~= sum_j beta[j] exp(-c_j u), u=d^2, d uniform [0,dmax]."""
    d = (np.arange(n_samp, dtype=np.float64) + 0.5) / n_samp * dmax
    u = d * d
    Amat = np.exp(-np.outer(u, c_pool))
    Bmat = np.exp(-np.outer(u, gammas))
    scale = Bmat.mean(axis=0)
    AtA = Amat.T @ Amat
    lam = ridge * np.trace(AtA) / len(c_pool)
    beta_n = np.linalg.solve(AtA + lam * np.eye(len(c_pool)), Amat.T @ (Bmat / scale))
    beta = beta_n * scale
    resid = np.abs(Amat @ beta - Bmat).max(axis=0) / scale
    return beta, float(resid.max())


RESID_TOL = float(os.environ.get("DENS_RESID_TOL", "1.5e-3"))


def _needed_set(pool, seeds, J):
    need = set(pool)
    for j in range(J - 1, 0, -1):
        if j in need and j not in seeds:
            need.add(j - 1)
    return need


def _plan(gammas, dmax):
    """Choose ladder, seed rungs, pooled subset, and fit beta.

    Searches seed patterns x pooled subsets minimizing a per-engine cost
    model (ACT: u + seed exps; DVE: squarings; PE: pooled rungs) subject
    to the host-side fit residual staying under RESID_TOL."""
    import itertools

    gammas = np.asarray(gammas, dtype=np.float64)
    gmin, gmax = float(gammas.min()), float(gammas.max())
    assert gmin > 0, "gammas must be positive"
    j_lo = int(np.floor(np.log2(gmin)))
    j_hi = int(np.ceil(np.log2(gmax)))
    j_hi = max(j_hi, j_lo)  # at least one rung
    if j_hi - j_lo > 15:  # cap ladder length; fit covers the rest
        j_lo = j_hi - 15
    c_full = 2.0 ** np.arange(j_lo, j_hi + 1)
    J = len(c_full)

    seed_opts = {tuple(range(0, J, SEED_EVERY))}
    for step in (3, 4, 5):
        for nk in (2, 3, 4):
            seed_opts.add(
                tuple(sorted({max(0, J - 1 - k * step) for k in range(nk)}))
            )
    best = None
    np_lo = min(6, NPOOL_MAX)
    resid_cache = {}
    for seeds in seed_opts:
        sset = set(seeds)
        for NP in range(np_lo, NPOOL_MAX + 1):
            for pool in itertools.combinations(range(J), min(NP, J)):
                need = _needed_set(pool, sset, J)
                if 0 in need and 0 not in sset:
                    continue  # chain bottom has no seed to start from
                if pool not in resid_cache:
                    _, resid_cache[pool] = _fit_beta(
                        gammas, c_full[list(pool)], dmax, n_samp=256
                    )
                r = resid_cache[pool]
                if r > RESID_TOL:
                    continue
                n_seed = len([s for s in seeds if s in need])
                n_sq = len(need) - n_seed
                pe_t = len(pool) * 1.35 + 1.2
                # u = d^2 goes to whichever of ACT/DVE balances better
                for u_act in (True, False):
                    act_t = (n_seed + (1 if u_act else 0)) * 1.95
                    dve_t = n_sq * 1.35 + (0 if u_act else 2.3)
                    score = max(act_t, dve_t, 0.9 * pe_t) + 0.1 * (
                        act_t + dve_t + pe_t
                    )
                    key = (score, r)
                    if best is None or key < best[0]:
                        best = (
                            key,
                            tuple(pool),
                            tuple(s for s in seeds if s in need),
                            "act" if u_act else "vector",
                        )
    assert best is not None, "no ladder subset met the fit tolerance"
    pool = list(best[1])
    seed_idx = list(best[2])
    u_engine = best[3]
    beta, resid = _fit_beta(gammas, c_full[pool], dmax)
    needed = _needed_set(pool, set(seed_idx), J)
    # slot layout: pooled rungs -> slots 0..NP-1 (ladder order); rest after
    slot_of = {}
    for s_, j in enumerate(pool):
        slot_of[j] = s_
    nxt = len(pool)
    for j in range(J):
        if j not in slot_of:
            slot_of[j] = nxt
            nxt += 1
    return {
        "c_full": tuple(float(c) for c in c_full),
        "seed_idx": tuple(j for j in seed_idx if j in needed),
        "slot_of": tuple(slot_of[j] for j in range(J)),
        "needed": tuple(sorted(needed)),
        "npool": len(pool),
        "u_engine": U_ENGINE if U_ENGINE != "auto" else u_engine,
        "beta": beta.astype(np.float32),  # (NP, S)
        "resid": resid,
    }


# ---------------------------------------------------------------- program


def _build_program(c_full, seed_idx, slot_of, npool, needed, u_engine):
    J = len(c_full)
    NP = npool
    seed_set = set(seed_idx)
    needed = set(needed)
    nc = bacc.Bacc("TRN2", target_bir_lowering=False, debug=False, num_devices=N_CORES)

    d_dram = nc.dram_tensor("d", [X, A], F32, kind="ExternalInput")
    wr_dram = nc.dram_tensor("wr", [X], F32, kind="ExternalInput")
    beta_dram = nc.dram_tensor("beta", [2 * NP * S], F32, kind="ExternalInput")
    wt_dram = nc.dram_tensor("wt", [S + 1, E], F32, kind="ExternalInput")
    epsn_dram = nc.dram_tensor("epsn", [S], F32, kind="ExternalInput")
    y_dram = nc.dram_tensor("y", [A, E], F32, kind="ExternalOutput")

    group_bounds = [0]
    for g in GROUPS:
        group_bounds.append(group_bounds[-1] + g)
    assert group_bounds[-1] == C, f"groups {GROUPS} must sum to {C}"

    with tile.TileContext(nc) as tc:
        with (
            tc.tile_pool(name="singles", bufs=1) as singles,
            tc.tile_pool(name="upool", bufs=int(os.environ.get("DENS_UBUFS", "4"))) as upool,
            tc.tile_pool(name="tpool", bufs=int(os.environ.get("DENS_TBUFS", "4"))) as tpool,
            tc.tile_pool(name="psum", bufs=1, space="PSUM") as psum,
        ):
            # ---- input loads (d pieces per compute group for pipelining;
            # first piece first so group-1 compute starts ASAP) ----
            d_sb = singles.tile([P, C, A], F32)
            d_src = d_dram.ap().rearrange("(p c) a -> p c a", p=P)
            for q in range(0, 2):
                lo, hi = group_bounds[q], group_bounds[q + 1]
                nc.sync.dma_start(out=d_sb[:, lo:hi, :], in_=d_src[:, lo:hi, :])
            wr_sb = singles.tile([P, C], F32)
            nc.sync.dma_start(out=wr_sb[:], in_=wr_dram.ap().rearrange("(p c) -> p c", p=P))
            wr_bf = singles.tile([P, C], BF16)
            nc.vector.tensor_copy(wr_bf[:], wr_sb[:])
            for q in range(2, len(group_bounds) - 1):
                lo, hi = group_bounds[q], group_bounds[q + 1]
                nc.sync.dma_start(out=d_sb[:, lo:hi, :], in_=d_src[:, lo:hi, :])

            # small tail constants behind the d pieces on the sync queue
            # (issuing them from nc.scalar stalls the ACT sequencer mid-loop)
            # beta rows flattened on partition 0: row j at [0, j*S:(j+1)*S]
            beta_sb = singles.tile([1, 2 * NP * S], F32)
            nc.sync.dma_start(out=beta_sb[:], in_=beta_dram.ap().unsqueeze(0))
            wt_sb = singles.tile([S + 1, E], F32)
            nc.sync.dma_start(out=wt_sb[:], in_=wt_dram.ap())
            epsn_sb = singles.tile([S, 1], F32)
            nc.sync.dma_start(out=epsn_sb[:], in_=epsn_dram.ap().unsqueeze(1))

            # ---- main loop: ladder + pooling ----
            # pooling is split: chunks [0, SPLIT) accumulate in psA, whose
            # copy + rank-1 interp updates overlap the last group's ladder;
            # only the small psB part stays on the post-loop critical path.
            # interp: pooled_s[s,a] = sum_j beta[j,s] * pooled[j*A+a] done as
            # 2*NP rank-1 matmuls (lhsT = beta row [1,S], rhs = pooled piece
            # [1,A]) accumulating straight into a [S,A] PSUM tile -- no
            # reshape/transpose of the flat pooled vector is ever needed.
            SPLIT = group_bounds[-2]
            pooled_psA = psum.tile([1, NP * A], F32)
            pooled_psB = psum.tile([1, NP * A], F32)
            pooled_sbA = singles.tile([1, NP * A], F32)
            interp_ps = psum.tile([S, A], F32)
            for q in range(len(group_bounds) - 1):
                c0, c1 = group_bounds[q], group_bounds[q + 1]
                gsz = c1 - c0
                u_g = upool.tile([P, gsz, A], BF16, tag="u")
                if u_engine == "gpsimd":
                    nc.gpsimd.tensor_mul(u_g[:], d_sb[:, c0:c1, :], d_sb[:, c0:c1, :])
                elif u_engine == "vector":
                    nc.vector.tensor_mul(u_g[:], d_sb[:, c0:c1, :], d_sb[:, c0:c1, :])
                else:
                    nc.scalar.activation(u_g[:], d_sb[:, c0:c1, :], AF.Square)
                # chunk-major layout: per-chunk slab [J, A] so the pooling
                # matmul rhs [NP*A] is contiguous (strided rhs costs ~320
                # PE cycles/call extra)
                t_g = tpool.tile([P, gsz, J, A], BF16, tag="t")
                for j in range(J):
                    if j not in needed:
                        continue
                    slot = slot_of[j]
                    if j in seed_set:
                        nc.scalar.activation(
                            t_g[:, :, slot, :], u_g[:], AF.Exp, scale=-c_full[j]
                        )
                    else:
                        prev = slot_of[j - 1]
                        eng = nc.gpsimd if j in GP_SLOTS else nc.vector
                        eng.tensor_mul(
                            t_g[:, :, slot, :], t_g[:, :, prev, :], t_g[:, :, prev, :]
                        )
                if c0 == SPLIT:
                    # A-part tail, issued after the last group's ladder ops
                    # so ACT runs the copy right after its final seeds and PE
                    # runs the interp updates before the (ladder-gated) B pools
                    nc.scalar.copy(pooled_sbA[:], pooled_psA[:])
                    for j in range(NP):
                        nc.tensor.matmul(
                            interp_ps[:],
                            beta_sb[:, j * S : (j + 1) * S],
                            pooled_sbA[:, j * A : (j + 1) * A],
                            start=(j == 0),
                            stop=False,
                            skip_group_check=True,
                        )
                for k in range(gsz):
                    c = c0 + k
                    in_a = c < SPLIT
                    nc.tensor.matmul(
                        pooled_psA[:, :] if in_a else pooled_psB[:, :],
                        wr_bf[:, c : c + 1],
                        t_g[:, k, 0:NP, :],
                        start=(c == 0 or c == SPLIT),
                        stop=(c == SPLIT - 1 or c == C - 1),
                    )

            # ---- tail (B part + combine) ----
            pooled_sbB = singles.tile([1, NP * A], F32)
            half = (NP * A) // 2
            nc.scalar.copy(pooled_sbB[:, :half], pooled_psB[:, :half])
            nc.vector.tensor_copy(pooled_sbB[:, half:], pooled_psB[:, half:])
            for j in range(NP):
                nc.tensor.matmul(
                    interp_ps[:],
                    beta_sb[:, (NP + j) * S : (NP + j + 1) * S],
                    pooled_sbB[:, j * A : (j + 1) * A],
                    start=False,
                    stop=(j == NP - 1),
                    skip_group_check=True,
                )

            # phi = ln(pooled_s + eps/norms); the "+ ln(norms)" term is folded
            # into the final matmul via a constant ones-row in phi and an
            # extra row sum_s ln(norms_s) W[e,s] in wt (host-computed)
            phi = singles.tile([S + 1, A], F32)
            nc.vector.memset(phi[S : S + 1, :], 1.0)
            nc.scalar.activation(
                phi[0:S, :], interp_ps[:], AF.Ln, bias=epsn_sb[:], scale=1.0
            )

            # final lift, split into halves so copy/DMA receipts overlap
            out_ps = psum.tile([A, E], F32)
            out_sb = singles.tile([A, E], F32)
            y_ap = y_dram.ap()
            for h in range(2):
                cs = slice(h * (E // 2), (h + 1) * (E // 2))
                nc.tensor.matmul(
                    out_ps[:, cs], phi[:], wt_sb[:, cs], start=True, stop=True
                )
                if h == 0:
                    nc.scalar.copy(out_sb[:, cs], out_ps[:, cs])
                    nc.sync.dma_start(out=y_ap[:, cs], in_=out_sb[:, cs])
                else:
                    nc.vector.tensor_copy(out_sb[:, cs], out_ps[:, cs])
                    nc.scalar.dma_start(out=y_ap[:, cs], in_=out_sb[:, cs])

    nc.compile()
    _merge_act_table_loads(nc)
    return nc


def _merge_act_table_loads(nc):
    """Exp, Ln and Square all live in the 'natural_log_exp_and_others' set,
    but the table-load pass picks per-function sets, emitting a ~2.7us table
    swap at every transition. Point every load at the combined set and drop
    the redundant reloads (keeping any that carry semaphore waits/updates)."""
    from concourse.hw_specs import get_activation_tables

    tables = list(get_activation_tables(nc.m.arch).items())
    combined_id = None
    for i, (name, funcs) in enumerate(tables):
        if name == "natural_log_exp_and_others":
            combined_id = i
    if combined_id is None:
        return
    needed = {AF.Exp, AF.Ln, AF.Square}
    if not needed <= tables[combined_id][1]:
        return
    for b in nc.main_func.blocks:
        seen = False
        keep = []
        for inst in b.instructions:
            if isinstance(inst, mybir.InstLoadActFuncSet):
                si = inst.sync_info
                has_sync = si is not None and (
                    len(si.on_wait) > 0 or len(si.on_update) > 0
                )
                inst.act_func_set_id = combined_id
                if seen and not has_sync:
                    continue  # redundant reload of the same set
                seen = True
            keep.append(inst)
        if len(keep) != len(b.instructions):
            b.instructions[:] = keep


# ---------------------------------------------------------------- entry


_CACHE = {}


def _get_program_and_plan(gammas, dmax):
    plan = _plan(gammas, dmax)
    key = (
        plan["c_full"], plan["seed_idx"], plan["slot_of"], plan["npool"],
        plan["needed"], plan["u_engine"],
    )
    if key not in _CACHE:
        _CACHE[key] = _build_program(
            plan["c_full"], plan["seed_idx"], plan["slot_of"], plan["npool"],
            plan["needed"], plan["u_engine"],
        )
    return _CACHE[key], plan


def _make_in_maps(wrho, distances, gammas, W, plan):
    wrho = np.ascontiguousarray(np.asarray(wrho, dtype=np.float32))
    distances = np.ascontiguousarray(np.asarray(distances, dtype=np.float32))
    gammas = np.asarray(gammas, dtype=np.float64)
    W = np.asarray(W, dtype=np.float32)
    assert wrho.shape == (B, X) and distances.shape == (B, X, A)
    assert gammas.shape == (S,) and W.shape == (E, S)
    norms = (np.pi / gammas) ** 1.5
    epsn = (EPS / norms).astype(np.float32)
    lnorms = 1.5 * np.log(np.pi / gammas)
    # wt row S carries sum_s ln(norms_s) W[e,s]; paired with a constant
    # ones-row in phi it adds the + ln(norms) term during the final matmul
    wt = np.ascontiguousarray(
        np.vstack([W.T.astype(np.float64), (lnorms @ W.T.astype(np.float64))[None, :]])
    ).astype(np.float32)
    # duplicated rows: the interp rank-1 updates sum the split A/B pooled
    # halves; flattened row-major (row j at [j*S:(j+1)*S])
    beta = np.ascontiguousarray(
        np.vstack([plan["beta"], plan["beta"]]).reshape(-1)
    )
    return [
        {
            "d": distances[b],
            "wr": wrho[b],
            "beta": beta,
            "wt": wt,
            "epsn": epsn,
        }
        for b in range(B)
    ]


def kernel(wrho, distances, gammas, W, **_unused):
    dmax = float(np.abs(np.asarray(distances)).max())
    nc, plan = _get_program_and_plan(gammas, max(dmax, 1e-6))
    in_maps = _make_in_maps(wrho, distances, gammas, W, plan)
    res = run_bass_kernel_spmd(nc, in_maps, core_ids=list(range(N_CORES)))
    return np.stack([res.results[b]["y"] for b in range(B)], axis=0)


def kernel_traced(wrho, distances, gammas, W):
    """Like kernel() but with NTFF tracing; returns (out, BassKernelResults)."""
    dmax = float(np.abs(np.asarray(distances)).max())
    nc, plan = _get_program_and_plan(gammas, max(dmax, 1e-6))
    in_maps = _make_in_maps(wrho, distances, gammas, W, plan)
    res = run_bass_kernel_spmd(nc, in_maps, core_ids=list(range(N_CORES)), trace=True)
    out = np.stack([res.results[b]["y"] for b in range(B)], axis=0)
    return out, res
